# revision 23
# baseline (speedup 1.0000x reference)
"""MultiHeadSectionAttentionImputer on 8 TRN2 NeuronCores (Bass/Tile).

Sharding: 2 head-groups x 4 key-shards. Core c handles heads
[6*(c//4), 6*(c//4)+6) and exist-keys [1536*(c%4), 1536*(c%4)+1536).
Each core:
  - projects its key shard to K,V (K = X_e @ Wk; V = X_e @ Wv with an
    appended ones column), its 6 heads only
  - projects the full missing set to Q for its 6 heads (Wq,bq pre-scaled
    by 1/sqrt(d_k) on host; bk dropped - it only shifts scores by a
    per-query constant, softmax-invariant and consistent across shards)
  - computes scoresT[key, query] per head with a fused 128-deep
    contraction: d' = [q-dims(64) | cooc-bias-dims(64)] so one matmul
    yields q.k/sqrt(dk) + mb.eb
  - exp() without max subtraction (scores bounded ~<60, safe in fp32)
  - attn @ [V | 1] accumulated over the 12 key chunks -> partial
    numerators (64 cols) + denominator per query
Host combines partials across the 4 key-shards of each head group
(exact softmax over all 6144 keys), adds bv, scatters into ehr.

Matmul inputs are fp16 (psum accumulates fp32); the attention weights
are bf16 (exp output needs fp32-like range; no max subtraction).
"""

import os
import sys
import numpy as np
from contextlib import ExitStack

sys.path.insert(0, "/opt/trn_rl_repo")

# problem constants (hardcoded; kernel.py must be self-contained)
H = 12          # total heads
DK = 64         # head dim
E = 768         # embed dim
TOTAL = H * DK  # 768
M = 2048        # missing sections
N = 6144        # existing sections
CORES = 8
HGROUPS = 2     # head groups (cores 0-3 -> heads 0-5, cores 4-7 -> 6-11)
NSHARDS = 4
HH = H // HGROUPS        # 6 heads per core
PP = HH // 2             # 3 head pairs per core
TT = HH * DK             # 384 projection cols per core
NLOC = N // NSHARDS      # 1536 keys per core
EC = E // 128            # 6 contraction chunks
NI = NLOC // 128         # 12 key chunks per core
MI = M // 128            # 16 query chunks

_CACHE = {}
LAST_EXEC_NS = None
LAST_TRACE_DIR = None


def _build():
    import concourse.bass as bass
    import concourse.tile as tile
    from concourse import bacc, mybir
    from collections import deque

    F32 = mybir.dt.float32
    FP16 = mybir.dt.float16
    BF16 = mybir.dt.bfloat16
    Exp = mybir.ActivationFunctionType.Exp

    nc = bacc.Bacc("TRN2", target_bir_lowering=False, debug=False)

    # ---- I/O (layouts chosen so every DMA is contiguous) ----
    xt_m = nc.dram_tensor("xt_m", [128, 4, EC, 512], FP16, kind="ExternalInput").ap()
    mbt = nc.dram_tensor("mbt", [HH * DK, M], FP16, kind="ExternalInput").ap()
    xt_e = nc.dram_tensor("xt_e", [128, 3, EC, 512], FP16, kind="ExternalInput").ap()
    ebt = nc.dram_tensor("ebt", [HH * DK, NLOC], FP16, kind="ExternalInput").ap()
    wq = nc.dram_tensor("wq", [128, PP, EC, 128], FP16, kind="ExternalInput").ap()
    wk = nc.dram_tensor("wk", [128, PP, EC, 128], FP16, kind="ExternalInput").ap()
    wv = nc.dram_tensor("wv", [128, EC, TT], FP16, kind="ExternalInput").ap()
    bq = nc.dram_tensor("bq", [128, PP], F32, kind="ExternalInput").ap()
    out_p = nc.dram_tensor("out_p", [HH, M, DK + 1], F32, kind="ExternalOutput").ap()

    with tile.TileContext(nc) as tc, ExitStack() as ctx:
        persist = ctx.enter_context(tc.tile_pool(name="persist", bufs=1))
        qpt_pool = ctx.enter_context(tc.tile_pool(name="qpt", bufs=4))
        attn_pool = ctx.enter_context(tc.tile_pool(name="attn", bufs=20))
        osb_pool = ctx.enter_context(tc.tile_pool(name="osb", bufs=12))
        proj_ps = ctx.enter_context(tc.tile_pool(name="proj_ps", bufs=2, space="PSUM"))
        sc_ps = ctx.enter_context(tc.tile_pool(name="sc_ps", bufs=2, space="PSUM"))
        av_ps = ctx.enter_context(tc.tile_pool(name="av_ps", bufs=2, space="PSUM"))

        # K'T per head [128, NLOC]: rows = k-dims | eb-dims (parity layout:
        # even head k at partitions 0:64, odd head k at 64:128 - avoids any
        # cross-partition copies; scores only need a consistent d' order)
        kpt = [persist.tile([128, NLOC], FP16, tag=f"kpt{h}", name=f"kpt{h}")
               for h in range(HH)]
        # V per key chunk [128, HH, DK+1] bf16, ones col at [., ., DK]
        vsb = [persist.tile([128, HH, DK + 1], BF16, tag=f"v{ni}", name=f"v{ni}")
               for ni in range(NI)]
        bq_sb = persist.tile([128, PP], F32, tag="bq")
        wk_big = persist.tile([128, PP, EC, 128], FP16, tag="wk")
        wq_big = persist.tile([128, PP, EC, 128], FP16, tag="wq")
        wv_big = persist.tile([128, EC, TT], FP16, tag="wv")
        xte_big = persist.tile([128, 3, EC, 512], FP16, tag="xte")
        xtm_big = persist.tile([128, 4, EC, 512], FP16, tag="xtm")
        q0_0 = qpt_pool.tile([128, M], FP16, tag="qpt", name="qpt0")
        q1_0 = qpt_pool.tile([128, M], FP16, tag="qpt", name="qpt1")

        # input DMAs: criticals (pair-0 / first key-third) on dedicated
        # queues, bulk remainder behind them
        nc.scalar.dma_start(bq_sb[:], bq)
        nc.sync.dma_start(wk_big[:, 0], wk[:, 0])
        nc.gpsimd.dma_start(q0_0[64:128, :], mbt[0:DK, :])
        nc.gpsimd.dma_start(q1_0[0:64, :], mbt[DK:2 * DK, :])
        nc.gpsimd.dma_start(wq_big[:, 0], wq[:, 0])
        nc.gpsimd.dma_start(kpt[0][64:128, :], ebt[0:DK, :])
        nc.gpsimd.dma_start(kpt[1][0:64, :], ebt[DK:2 * DK, :])
        # kt0's key-third split across sync+gpsimd so no single ~100GB/s
        # queue carries the whole 1.5MB on the critical path
        nc.sync.dma_start(xte_big[:, 0, 0:3], xt_e[:, 0, 0:3])
        nc.gpsimd.dma_start(xte_big[:, 0, 3:EC], xt_e[:, 0, 3:EC])
        nc.scalar.dma_start(xtm_big[:, 0], xt_m[:, 0])
        nc.scalar.dma_start(xtm_big[:, 1], xt_m[:, 1])
        nc.sync.dma_start(xte_big[:, 1], xt_e[:, 1])
        nc.gpsimd.dma_start(wv_big[:], wv[:])
        nc.scalar.dma_start(xtm_big[:, 2:4], xt_m[:, 2:4])
        nc.sync.dma_start(xte_big[:, 2], xt_e[:, 2])
        nc.gpsimd.dma_start(wk_big[:, 1:PP], wk[:, 1:PP])
        nc.gpsimd.dma_start(wq_big[:, 1:PP], wq[:, 1:PP])

        def emit_scores_exp_half(h, qt, ni, half, at):
            """scoresT half [128 keys, 1024 queries] + exp into attnT.
            [128,1024] scores psum tiles double-buffer (2 banks each) so
            ACT runs exp back-to-back with no psum-free wait."""
            ps = sc_ps.tile([128, 1024], F32, tag="sc", name="sc_ps_t")
            mo = half * 1024
            for mj in range(2):
                nc.tensor.matmul(
                    ps[:, mj * 512:(mj + 1) * 512],
                    lhsT=kpt[h][:, ni * 128:(ni + 1) * 128],
                    rhs=qt[:, mo + mj * 512:mo + (mj + 1) * 512],
                    start=True, stop=True)
            nc.scalar.activation(at[:, mo:mo + 1024], ps[:], Exp)

        def emit_av(h, attns, g):
            """out chunks [128 queries, DK+1], mi in [2g, 2g+2); the bf16
            attnT chunk is the stationary operand (fast weight load)."""
            for mi in range(2 * g, 2 * g + 2):
                ps = av_ps.tile([128, DK + 1], F32, tag="av", name="av_ps_t")
                for ni in range(NI):
                    nc.tensor.matmul(
                        ps[:], lhsT=attns[ni][:, mi * 128:(mi + 1) * 128],
                        rhs=vsb[ni][:, h, :],
                        start=(ni == 0), stop=(ni == NI - 1))
                ot = osb_pool.tile([128, DK + 1], F32, tag="osb", name="osb_t")
                nc.vector.tensor_copy(ot[:], ps[:])
                nc.sync.dma_start(out_p[h, mi * 128:(mi + 1) * 128, :], ot[:])

        # ---- unit-queue scheduler ----
        units = deque()
        qts = {0: q0_0, 1: q1_0}
        pieces = {0: set()}  # pair -> done piece ids (k0..k2, q0..q3)

        def emit_qt_quarter(p, mh, q0, q1):
            ps = proj_ps.tile([128, 512], F32, tag="proj", name="proj_qt")
            for ec in range(EC):
                nc.tensor.matmul(ps[:], lhsT=wq_big[:, p, ec, :],
                                 rhs=xtm_big[:, mh, ec, :],
                                 start=(ec == 0), stop=(ec == EC - 1))
            mo = mh * 512
            nc.vector.tensor_scalar_add(
                q0[0:64, mo:mo + 512], ps[0:64, :], bq_sb[0:64, p:p + 1])
            nc.vector.tensor_scalar_add(
                q1[64:128, mo:mo + 512], ps[64:128, :], bq_sb[64:128, p:p + 1])

        def qt_unit(p, mh):
            def f():
                pieces.setdefault(p, set()).add(f"q{mh}")
                q0, q1 = qts.get(2 * p), qts.get(2 * p + 1)
                if q0 is None:
                    q0 = qpt_pool.tile([128, M], FP16, tag="qpt", name=f"qpt{2*p}")
                    q1 = qpt_pool.tile([128, M], FP16, tag="qpt", name=f"qpt{2*p+1}")
                    h0, h1 = 2 * p, 2 * p + 1
                    nc.sync.dma_start(q0[64:128, :], mbt[h0 * DK:(h0 + 1) * DK, :])
                    nc.sync.dma_start(q1[0:64, :], mbt[h1 * DK:(h1 + 1) * DK, :])
                    qts[2 * p], qts[2 * p + 1] = q0, q1
                emit_qt_quarter(p, mh, q0, q1)
            return (1.35, f)

        def emit_kt_third(p, t):
            h0, h1 = 2 * p, 2 * p + 1
            lo = t * 512
            ps = proj_ps.tile([128, 512], F32, tag="proj", name="proj_kt")
            for ec in range(EC):
                nc.tensor.matmul(ps[:], lhsT=wk_big[:, p, ec, :],
                                 rhs=xte_big[:, t, ec, :],
                                 start=(ec == 0), stop=(ec == EC - 1))
            nc.vector.tensor_copy(kpt[h0][0:64, lo:lo + 512], ps[0:64, :])
            nc.vector.tensor_copy(kpt[h1][64:128, lo:lo + 512], ps[64:128, :])
            if t == 0 and p > 0:
                nc.sync.dma_start(kpt[h0][64:128, :], ebt[h0 * DK:(h0 + 1) * DK, :])
                nc.sync.dma_start(kpt[h1][0:64, :], ebt[h1 * DK:(h1 + 1) * DK, :])

        def kt_unit(p, t):
            def f():
                pieces.setdefault(p, set()).add(f"k{t}")
                emit_kt_third(p, t)
            return (1.35, f)

        def v_unit(ni):
            def f():
                ps = proj_ps.tile([128, TT], F32, tag="proj", name="proj_v")
                t, off = divmod(ni, 4)
                for ec in range(EC):
                    nc.tensor.matmul(
                        ps[:], lhsT=xte_big[:, t, ec, off * 128:(off + 1) * 128],
                        rhs=wv_big[:, ec, :], start=(ec == 0), stop=(ec == EC - 1))
                nc.vector.tensor_copy(
                    vsb[ni][:, :, 0:DK], ps[:].rearrange("p (h d) -> p h d", d=DK))
                nc.vector.memset(vsb[ni][:, :, DK], 1.0)
            return (1.0, f)

        def av_unit(h, attns, g):
            def f():
                emit_av(h, attns, g)
            return (0.8, f)

        def pump(budget):
            while units and budget > 0:
                c, f = units.popleft()
                f()
                budget -= c

        # minimal head-0 critical path up front: kt third0 + qt q0/q1
        emit_kt_third(0, 0)
        pieces[0].add("k0")
        qt_unit(0, 0)[1]()
        qt_unit(0, 1)[1]()
        units.append(kt_unit(0, 1))
        units.append(kt_unit(0, 2))
        units.append(qt_unit(0, 2))
        units.append(qt_unit(0, 3))
        for ni in range(NI):
            units.append(v_unit(ni))

        def need(p, ni, half):
            req = {f"k{ni // 4}", f"q{2 * half}", f"q{2 * half + 1}"}
            while not req <= pieces.get(p, set()):
                c, f = units.popleft()
                f()

        slot = 0
        for h in range(HH):
            p = h // 2
            if h % 2 == 1 and p + 1 <= PP - 1:
                # next pair's projections jump the queue (front) so the
                # even-head boundary never force-drains a big batch
                for mh in range(3, -1, -1):
                    units.appendleft(qt_unit(p + 1, mh))
                for t in range(2, -1, -1):
                    units.appendleft(kt_unit(p + 1, t))
            attns = [attn_pool.tile([128, M], BF16, tag="attn",
                                    name=f"attn_{h}_{ni}") for ni in range(NI)]
            for half in range(2):
                for ni in range(NI):
                    need(p, ni, half)
                    emit_scores_exp_half(h, qts[h], ni, half, attns[ni])
                    pump(2.0 if slot < 16 else 0.7)
                    slot += 1
                # av groups for mi chunks covered by this half can go
                # into the queue now (g<4 reads attnT cols 0:1024 only)
                gs = range(4) if half == 0 else range(4, 8)
                for g in gs:
                    units.append(av_unit(h, attns, g))
            qts[h] = None  # release the qpt slot
        while units:
            c, f = units.popleft()
            f()

    nc.compile()
    return nc


def _get_nc():
    if "nc" not in _CACHE:
        _CACHE["nc"] = _build()
    return _CACHE["nc"]


def kernel(**inputs):
    global LAST_EXEC_NS, LAST_TRACE_DIR
    from concourse.bass_utils import run_bass_kernel_spmd

    ehr = np.asarray(inputs["ehr_embeddings"], dtype=np.float32)
    mi = np.asarray(inputs["missing_indices"]).astype(np.int64)
    ei = np.asarray(inputs["exist_indices"]).astype(np.int64)
    Wq = np.asarray(inputs["Wq"], dtype=np.float32)
    Wk = np.asarray(inputs["Wk"], dtype=np.float32)
    Wv = np.asarray(inputs["Wv"], dtype=np.float32)
    bq = np.asarray(inputs["bq"], dtype=np.float32)
    bv = np.asarray(inputs["bv"], dtype=np.float32)
    cooc = np.asarray(inputs["cooc_bias"], dtype=np.float32)

    scale = 1.0 / np.sqrt(np.float32(DK))

    def fold(a):  # [E, F] -> [128, EC, F]
        return a.reshape(EC, 128, a.shape[1]).transpose(1, 0, 2)

    def wfold(a):  # [E, TT] -> [128, PP, EC, 128] (pair-col major)
        return np.ascontiguousarray(
            fold(a).reshape(128, EC, PP, 128).transpose(0, 2, 1, 3))

    missing_emb = ehr[mi]                       # [M, E]
    xt_m = np.ascontiguousarray(
        fold(missing_emb.T.astype(np.float16))
        .reshape(128, EC, 4, 512).transpose(0, 2, 1, 3))  # [128, 4, EC, 512]
    wq_all = (Wq * scale).astype(np.float16)
    wk_all = Wk.astype(np.float16)
    wv_all = Wv.astype(np.float16)
    mbt_all = cooc[:, mi, :].transpose(0, 2, 1).reshape(H * DK, M).astype(np.float16)
    bq_all = (bq * scale).astype(np.float32)

    in_maps = []
    for c in range(CORES):
        hg, ns = c // NSHARDS, c % NSHARDS
        hsl = slice(hg * TT, (hg + 1) * TT)
        eic = ei[ns * NLOC:(ns + 1) * NLOC]
        xte_f = fold(ehr[eic].T.astype(np.float16))  # [128, EC, NLOC]
        xt_e = np.ascontiguousarray(
            xte_f.reshape(128, EC, 3, 512).transpose(0, 2, 1, 3))
        ebt = np.ascontiguousarray(
            cooc[hg * HH:(hg + 1) * HH, eic, :].transpose(0, 2, 1)
            .reshape(HH * DK, NLOC).astype(np.float16))
        in_maps.append({
            "xt_m": xt_m,
            "mbt": np.ascontiguousarray(mbt_all[hsl]),
            "xt_e": xt_e, "ebt": ebt,
            "wq": wfold(wq_all[:, hsl]),
            "wk": wfold(wk_all[:, hsl]),
            "wv": np.ascontiguousarray(fold(wv_all[:, hsl])),
            "bq": np.ascontiguousarray(bq_all[hsl].reshape(PP, 128).T),
        })

    nc = _get_nc()
    kwargs = {}
    if os.environ.get("KERNEL_TRACE") == "1":
        import tempfile
        LAST_TRACE_DIR = tempfile.mkdtemp(prefix="kern_trace_")
        kwargs = {"trace": True, "tmpdir": LAST_TRACE_DIR}
        try:
            import ntff_shim
            ntff_shim.install()
        except ImportError:
            pass
    res = run_bass_kernel_spmd(nc, in_maps, list(range(CORES)), **kwargs)
    LAST_EXEC_NS = res.exec_time_ns

    # ---- host combine (exact softmax across the 4 key shards) ----
    num = np.zeros((H, M, DK), dtype=np.float64)
    den = np.zeros((H, M), dtype=np.float64)
    for c in range(CORES):
        hg = c // NSHARDS
        op = res.results[c]["out_p"].astype(np.float64)  # [HH, M, DK+1]
        num[hg * HH:(hg + 1) * HH] += op[:, :, :DK]
        den[hg * HH:(hg + 1) * HH] += op[:, :, DK]
    out = num / den[:, :, None]                          # [H, M, DK]
    out = out.transpose(1, 0, 2).reshape(M, TOTAL) + bv.astype(np.float64)
    result = ehr.copy()
    result[mi] = out.astype(np.float32)
    return result


# revision 24
# speedup vs baseline: 1.0120x; 1.0120x over previous
"""MultiHeadSectionAttentionImputer on 8 TRN2 NeuronCores (Bass/Tile).

Sharding: 2 head-groups x 4 key-shards. Core c handles heads
[6*(c//4), 6*(c//4)+6) and exist-keys [1536*(c%4), 1536*(c%4)+1536).
Each core:
  - projects its key shard to K,V (K = X_e @ Wk; V = X_e @ Wv with an
    appended ones column), its 6 heads only
  - projects the full missing set to Q for its 6 heads (Wq,bq pre-scaled
    by 1/sqrt(d_k) on host; bk dropped - it only shifts scores by a
    per-query constant, softmax-invariant and consistent across shards)
  - computes scoresT[key, query] per head with a fused 128-deep
    contraction: d' = [q-dims(64) | cooc-bias-dims(64)] so one matmul
    yields q.k/sqrt(dk) + mb.eb
  - exp() without max subtraction (scores bounded ~<60, safe in fp32)
  - attn @ [V | 1] accumulated over the 12 key chunks -> partial
    numerators (64 cols) + denominator per query
Host combines partials across the 4 key-shards of each head group
(exact softmax over all 6144 keys), adds bv, scatters into ehr.

Matmul inputs are fp16 (psum accumulates fp32); the attention weights
are bf16 (exp output needs fp32-like range; no max subtraction).
"""

import os
import sys
import numpy as np
from contextlib import ExitStack

sys.path.insert(0, "/opt/trn_rl_repo")

# problem constants (hardcoded; kernel.py must be self-contained)
H = 12          # total heads
DK = 64         # head dim
E = 768         # embed dim
TOTAL = H * DK  # 768
M = 2048        # missing sections
N = 6144        # existing sections
CORES = 8
HGROUPS = 2     # head groups (cores 0-3 -> heads 0-5, cores 4-7 -> 6-11)
NSHARDS = 4
HH = H // HGROUPS        # 6 heads per core
PP = HH // 2             # 3 head pairs per core
TT = HH * DK             # 384 projection cols per core
NLOC = N // NSHARDS      # 1536 keys per core
EC = E // 128            # 6 contraction chunks
NI = NLOC // 128         # 12 key chunks per core
MI = M // 128            # 16 query chunks

_CACHE = {}
LAST_EXEC_NS = None
LAST_TRACE_DIR = None


def _build():
    import concourse.bass as bass
    import concourse.tile as tile
    from concourse import bacc, mybir
    from collections import deque

    F32 = mybir.dt.float32
    FP16 = mybir.dt.float16
    BF16 = mybir.dt.bfloat16
    Exp = mybir.ActivationFunctionType.Exp

    nc = bacc.Bacc("TRN2", target_bir_lowering=False, debug=False)

    # ---- I/O (layouts chosen so every DMA is contiguous) ----
    xt_m = nc.dram_tensor("xt_m", [128, 4, EC, 512], FP16, kind="ExternalInput").ap()
    mbt = nc.dram_tensor("mbt", [HH * DK, M], FP16, kind="ExternalInput").ap()
    xt_e = nc.dram_tensor("xt_e", [128, 3, EC, 512], FP16, kind="ExternalInput").ap()
    ebt = nc.dram_tensor("ebt", [HH * DK, NLOC], FP16, kind="ExternalInput").ap()
    wq = nc.dram_tensor("wq", [128, PP, EC, 128], FP16, kind="ExternalInput").ap()
    wk = nc.dram_tensor("wk", [128, PP, EC, 128], FP16, kind="ExternalInput").ap()
    wv = nc.dram_tensor("wv", [128, EC, TT], FP16, kind="ExternalInput").ap()
    bq = nc.dram_tensor("bq", [128, PP], F32, kind="ExternalInput").ap()
    out_p = nc.dram_tensor("out_p", [HH, M, DK + 1], F32, kind="ExternalOutput").ap()

    with tile.TileContext(nc) as tc, ExitStack() as ctx:
        persist = ctx.enter_context(tc.tile_pool(name="persist", bufs=1))
        qpt_pool = ctx.enter_context(tc.tile_pool(name="qpt", bufs=4))
        attn_pool = ctx.enter_context(tc.tile_pool(name="attn", bufs=20))
        osb_pool = ctx.enter_context(tc.tile_pool(name="osb", bufs=12))
        proj_ps = ctx.enter_context(tc.tile_pool(name="proj_ps", bufs=2, space="PSUM"))
        sc_ps = ctx.enter_context(tc.tile_pool(name="sc_ps", bufs=2, space="PSUM"))
        av_ps = ctx.enter_context(tc.tile_pool(name="av_ps", bufs=2, space="PSUM"))

        # K'T per head [128, NLOC]: rows = k-dims | eb-dims (parity layout:
        # even head k at partitions 0:64, odd head k at 64:128 - avoids any
        # cross-partition copies; scores only need a consistent d' order)
        kpt = [persist.tile([128, NLOC], FP16, tag=f"kpt{h}", name=f"kpt{h}")
               for h in range(HH)]
        # V per key chunk [128, HH, DK+1] bf16, ones col at [., ., DK]
        vsb = [persist.tile([128, HH, DK + 1], BF16, tag=f"v{ni}", name=f"v{ni}")
               for ni in range(NI)]
        bq_sb = persist.tile([128, PP], F32, tag="bq")
        wk_big = persist.tile([128, PP, EC, 128], FP16, tag="wk")
        wq_big = persist.tile([128, PP, EC, 128], FP16, tag="wq")
        wv_big = persist.tile([128, EC, TT], FP16, tag="wv")
        xte_big = persist.tile([128, 3, EC, 512], FP16, tag="xte")
        xtm_big = persist.tile([128, 4, EC, 512], FP16, tag="xtm")
        q0_0 = qpt_pool.tile([128, M], FP16, tag="qpt", name="qpt0")
        q1_0 = qpt_pool.tile([128, M], FP16, tag="qpt", name="qpt1")

        # input DMAs: criticals (pair-0 / first key-third) on dedicated
        # queues, bulk remainder behind them
        nc.scalar.dma_start(bq_sb[:], bq)
        nc.sync.dma_start(wk_big[:, 0], wk[:, 0])
        nc.gpsimd.dma_start(q0_0[64:128, :], mbt[0:DK, :])
        nc.gpsimd.dma_start(q1_0[0:64, :], mbt[DK:2 * DK, :])
        nc.gpsimd.dma_start(wq_big[:, 0], wq[:, 0])
        nc.gpsimd.dma_start(kpt[0][64:128, :], ebt[0:DK, :])
        nc.gpsimd.dma_start(kpt[1][0:64, :], ebt[DK:2 * DK, :])
        # kt0's key-third split across sync+gpsimd so no single ~100GB/s
        # queue carries the whole 1.5MB on the critical path
        nc.sync.dma_start(xte_big[:, 0, 0:3], xt_e[:, 0, 0:3])
        nc.gpsimd.dma_start(xte_big[:, 0, 3:EC], xt_e[:, 0, 3:EC])
        nc.scalar.dma_start(xtm_big[:, 0], xt_m[:, 0])
        nc.scalar.dma_start(xtm_big[:, 1], xt_m[:, 1])
        nc.sync.dma_start(xte_big[:, 1], xt_e[:, 1])
        nc.gpsimd.dma_start(wv_big[:], wv[:])
        nc.scalar.dma_start(xtm_big[:, 2:4], xt_m[:, 2:4])
        nc.sync.dma_start(xte_big[:, 2], xt_e[:, 2])
        nc.gpsimd.dma_start(wk_big[:, 1:PP], wk[:, 1:PP])
        nc.gpsimd.dma_start(wq_big[:, 1:PP], wq[:, 1:PP])

        def emit_scores_exp_half(h, qt, ni, half, at):
            """scoresT half [128 keys, 1024 queries] + exp into attnT.
            [128,1024] scores psum tiles double-buffer (2 banks each) so
            ACT runs exp back-to-back with no psum-free wait."""
            ps = sc_ps.tile([128, 1024], F32, tag="sc", name="sc_ps_t")
            mo = half * 1024
            for mj in range(2):
                nc.tensor.matmul(
                    ps[:, mj * 512:(mj + 1) * 512],
                    lhsT=kpt[h][:, ni * 128:(ni + 1) * 128],
                    rhs=qt[:, mo + mj * 512:mo + (mj + 1) * 512],
                    start=True, stop=True)
            nc.scalar.activation(at[:, mo:mo + 1024], ps[:], Exp)

        def emit_av(h, attns, g):
            """out chunks [128 queries, DK+1], mi in [2g, 2g+2); the bf16
            attnT chunk is the stationary operand (fast weight load)."""
            for mi in range(2 * g, 2 * g + 2):
                ps = av_ps.tile([128, DK + 1], F32, tag="av", name="av_ps_t")
                for ni in range(NI):
                    nc.tensor.matmul(
                        ps[:], lhsT=attns[ni][:, mi * 128:(mi + 1) * 128],
                        rhs=vsb[ni][:, h, :],
                        start=(ni == 0), stop=(ni == NI - 1))
                ot = osb_pool.tile([128, DK + 1], F32, tag="osb", name="osb_t")
                nc.vector.tensor_copy(ot[:], ps[:])
                nc.sync.dma_start(out_p[h, mi * 128:(mi + 1) * 128, :], ot[:])

        # ---- unit-queue scheduler ----
        units = deque()
        qts = {0: q0_0, 1: q1_0}
        pieces = {0: set()}  # pair -> done piece ids (k0..k2, q0..q3)

        def emit_qt_quarter(p, mh, q0, q1):
            ps = proj_ps.tile([128, 512], F32, tag="proj", name="proj_qt")
            for ec in range(EC):
                nc.tensor.matmul(ps[:], lhsT=wq_big[:, p, ec, :],
                                 rhs=xtm_big[:, mh, ec, :],
                                 start=(ec == 0), stop=(ec == EC - 1))
            mo = mh * 512
            nc.vector.tensor_scalar_add(
                q0[0:64, mo:mo + 512], ps[0:64, :], bq_sb[0:64, p:p + 1])
            nc.vector.tensor_scalar_add(
                q1[64:128, mo:mo + 512], ps[64:128, :], bq_sb[64:128, p:p + 1])

        def qt_unit(p, mh):
            def f():
                pieces.setdefault(p, set()).add(f"q{mh}")
                q0, q1 = qts.get(2 * p), qts.get(2 * p + 1)
                if q0 is None:
                    q0 = qpt_pool.tile([128, M], FP16, tag="qpt", name=f"qpt{2*p}")
                    q1 = qpt_pool.tile([128, M], FP16, tag="qpt", name=f"qpt{2*p+1}")
                    h0, h1 = 2 * p, 2 * p + 1
                    nc.sync.dma_start(q0[64:128, :], mbt[h0 * DK:(h0 + 1) * DK, :])
                    nc.sync.dma_start(q1[0:64, :], mbt[h1 * DK:(h1 + 1) * DK, :])
                    qts[2 * p], qts[2 * p + 1] = q0, q1
                emit_qt_quarter(p, mh, q0, q1)
            return (1.35, f)

        def emit_kt_third(p, t):
            h0, h1 = 2 * p, 2 * p + 1
            lo = t * 512
            ps = proj_ps.tile([128, 512], F32, tag="proj", name="proj_kt")
            for ec in range(EC):
                nc.tensor.matmul(ps[:], lhsT=wk_big[:, p, ec, :],
                                 rhs=xte_big[:, t, ec, :],
                                 start=(ec == 0), stop=(ec == EC - 1))
            nc.vector.tensor_copy(kpt[h0][0:64, lo:lo + 512], ps[0:64, :])
            nc.vector.tensor_copy(kpt[h1][64:128, lo:lo + 512], ps[64:128, :])
            if t == 0 and p > 0:
                nc.sync.dma_start(kpt[h0][64:128, :], ebt[h0 * DK:(h0 + 1) * DK, :])
                nc.sync.dma_start(kpt[h1][0:64, :], ebt[h1 * DK:(h1 + 1) * DK, :])

        def kt_unit(p, t):
            def f():
                pieces.setdefault(p, set()).add(f"k{t}")
                emit_kt_third(p, t)
            return (1.35, f)

        def v_unit(ni):
            def f():
                ps = proj_ps.tile([128, TT], F32, tag="proj", name="proj_v")
                t, off = divmod(ni, 4)
                for ec in range(EC):
                    nc.tensor.matmul(
                        ps[:], lhsT=xte_big[:, t, ec, off * 128:(off + 1) * 128],
                        rhs=wv_big[:, ec, :], start=(ec == 0), stop=(ec == EC - 1))
                nc.vector.tensor_copy(
                    vsb[ni][:, :, 0:DK], ps[:].rearrange("p (h d) -> p h d", d=DK))
                nc.vector.memset(vsb[ni][:, :, DK], 1.0)
            return (1.0, f)

        def av_unit(h, attns, g):
            def f():
                emit_av(h, attns, g)
            return (0.8, f)

        def pump(budget):
            while units and budget > 0:
                c, f = units.popleft()
                f()
                budget -= c

        # minimal head-0 critical path up front: kt third0 + qt q0/q1
        emit_kt_third(0, 0)
        pieces[0].add("k0")
        qt_unit(0, 0)[1]()
        qt_unit(0, 1)[1]()
        units.append(kt_unit(0, 1))
        units.append(kt_unit(0, 2))
        units.append(qt_unit(0, 2))
        units.append(qt_unit(0, 3))
        for ni in range(NI):
            units.append(v_unit(ni))

        def need(p, ni, half):
            req = {f"k{ni // 4}", f"q{2 * half}", f"q{2 * half + 1}"}
            while not req <= pieces.get(p, set()):
                c, f = units.popleft()
                f()

        slot = 0
        for h in range(HH):
            p = h // 2
            if h % 2 == 1 and p + 1 <= PP - 1:
                # next pair's projections jump the queue (front) so the
                # even-head boundary never force-drains a big batch
                for mh in range(3, -1, -1):
                    units.appendleft(qt_unit(p + 1, mh))
                for t in range(2, -1, -1):
                    units.appendleft(kt_unit(p + 1, t))
            attns = [attn_pool.tile([128, M], BF16, tag="attn",
                                    name=f"attn_{h}_{ni}") for ni in range(NI)]
            for half in range(2):
                for ni in range(NI):
                    need(p, ni, half)
                    emit_scores_exp_half(h, qts[h], ni, half, attns[ni])
                    pump(2.0 if slot < 24 else 0.75)
                    slot += 1
                # av groups for mi chunks covered by this half can go
                # into the queue now (g<4 reads attnT cols 0:1024 only)
                gs = range(4) if half == 0 else range(4, 8)
                for g in gs:
                    units.append(av_unit(h, attns, g))
            qts[h] = None  # release the qpt slot
        while units:
            c, f = units.popleft()
            f()

    nc.compile()
    return nc


def _get_nc():
    if "nc" not in _CACHE:
        _CACHE["nc"] = _build()
    return _CACHE["nc"]


def kernel(**inputs):
    global LAST_EXEC_NS, LAST_TRACE_DIR
    from concourse.bass_utils import run_bass_kernel_spmd

    ehr = np.asarray(inputs["ehr_embeddings"], dtype=np.float32)
    mi = np.asarray(inputs["missing_indices"]).astype(np.int64)
    ei = np.asarray(inputs["exist_indices"]).astype(np.int64)
    Wq = np.asarray(inputs["Wq"], dtype=np.float32)
    Wk = np.asarray(inputs["Wk"], dtype=np.float32)
    Wv = np.asarray(inputs["Wv"], dtype=np.float32)
    bq = np.asarray(inputs["bq"], dtype=np.float32)
    bv = np.asarray(inputs["bv"], dtype=np.float32)
    cooc = np.asarray(inputs["cooc_bias"], dtype=np.float32)

    scale = 1.0 / np.sqrt(np.float32(DK))

    def fold(a):  # [E, F] -> [128, EC, F]
        return a.reshape(EC, 128, a.shape[1]).transpose(1, 0, 2)

    def wfold(a):  # [E, TT] -> [128, PP, EC, 128] (pair-col major)
        return np.ascontiguousarray(
            fold(a).reshape(128, EC, PP, 128).transpose(0, 2, 1, 3))

    missing_emb = ehr[mi]                       # [M, E]
    xt_m = np.ascontiguousarray(
        fold(missing_emb.T.astype(np.float16))
        .reshape(128, EC, 4, 512).transpose(0, 2, 1, 3))  # [128, 4, EC, 512]
    wq_all = (Wq * scale).astype(np.float16)
    wk_all = Wk.astype(np.float16)
    wv_all = Wv.astype(np.float16)
    mbt_all = cooc[:, mi, :].transpose(0, 2, 1).reshape(H * DK, M).astype(np.float16)
    bq_all = (bq * scale).astype(np.float32)

    in_maps = []
    for c in range(CORES):
        hg, ns = c // NSHARDS, c % NSHARDS
        hsl = slice(hg * TT, (hg + 1) * TT)
        eic = ei[ns * NLOC:(ns + 1) * NLOC]
        xte_f = fold(ehr[eic].T.astype(np.float16))  # [128, EC, NLOC]
        xt_e = np.ascontiguousarray(
            xte_f.reshape(128, EC, 3, 512).transpose(0, 2, 1, 3))
        ebt = np.ascontiguousarray(
            cooc[hg * HH:(hg + 1) * HH, eic, :].transpose(0, 2, 1)
            .reshape(HH * DK, NLOC).astype(np.float16))
        in_maps.append({
            "xt_m": xt_m,
            "mbt": np.ascontiguousarray(mbt_all[hsl]),
            "xt_e": xt_e, "ebt": ebt,
            "wq": wfold(wq_all[:, hsl]),
            "wk": wfold(wk_all[:, hsl]),
            "wv": np.ascontiguousarray(fold(wv_all[:, hsl])),
            "bq": np.ascontiguousarray(bq_all[hsl].reshape(PP, 128).T),
        })

    nc = _get_nc()
    kwargs = {}
    if os.environ.get("KERNEL_TRACE") == "1":
        import tempfile
        LAST_TRACE_DIR = tempfile.mkdtemp(prefix="kern_trace_")
        kwargs = {"trace": True, "tmpdir": LAST_TRACE_DIR}
        try:
            import ntff_shim
            ntff_shim.install()
        except ImportError:
            pass
    res = run_bass_kernel_spmd(nc, in_maps, list(range(CORES)), **kwargs)
    LAST_EXEC_NS = res.exec_time_ns

    # ---- host combine (exact softmax across the 4 key shards) ----
    num = np.zeros((H, M, DK), dtype=np.float64)
    den = np.zeros((H, M), dtype=np.float64)
    for c in range(CORES):
        hg = c // NSHARDS
        op = res.results[c]["out_p"].astype(np.float64)  # [HH, M, DK+1]
        num[hg * HH:(hg + 1) * HH] += op[:, :, :DK]
        den[hg * HH:(hg + 1) * HH] += op[:, :, DK]
    out = num / den[:, :, None]                          # [H, M, DK]
    out = out.transpose(1, 0, 2).reshape(M, TOTAL) + bv.astype(np.float64)
    result = ehr.copy()
    result[mi] = out.astype(np.float32)
    return result


# revision 25
# speedup vs baseline: 1.0285x; 1.0163x over previous
"""MultiHeadSectionAttentionImputer on 8 TRN2 NeuronCores (Bass/Tile).

Sharding: 2 head-groups x 4 key-shards. Core c handles heads
[6*(c//4), 6*(c//4)+6) and exist-keys [1536*(c%4), 1536*(c%4)+1536).
Each core:
  - projects its key shard to K,V (K = X_e @ Wk; V = X_e @ Wv with an
    appended ones column), its 6 heads only
  - projects the full missing set to Q for its 6 heads (Wq,bq pre-scaled
    by 1/sqrt(d_k) on host; bk dropped - it only shifts scores by a
    per-query constant, softmax-invariant and consistent across shards)
  - computes scoresT[key, query] per head with a fused 128-deep
    contraction: d' = [q-dims(64) | cooc-bias-dims(64)] so one matmul
    yields q.k/sqrt(dk) + mb.eb
  - exp() without max subtraction (scores bounded ~<60, safe in fp32)
  - attn @ [V | 1] accumulated over the 12 key chunks -> partial
    numerators (64 cols) + denominator per query
Host combines partials across the 4 key-shards of each head group
(exact softmax over all 6144 keys), adds bv, scatters into ehr.

Matmul inputs are fp16 (psum accumulates fp32); the attention weights
are bf16 (exp output needs fp32-like range; no max subtraction).
"""

import os
import sys
import numpy as np
from contextlib import ExitStack

sys.path.insert(0, "/opt/trn_rl_repo")

# problem constants (hardcoded; kernel.py must be self-contained)
H = 12          # total heads
DK = 64         # head dim
E = 768         # embed dim
TOTAL = H * DK  # 768
M = 2048        # missing sections
N = 6144        # existing sections
CORES = 8
HGROUPS = 2     # head groups (cores 0-3 -> heads 0-5, cores 4-7 -> 6-11)
NSHARDS = 4
HH = H // HGROUPS        # 6 heads per core
PP = HH // 2             # 3 head pairs per core
TT = HH * DK             # 384 projection cols per core
NLOC = N // NSHARDS      # 1536 keys per core
EC = E // 128            # 6 contraction chunks
NI = NLOC // 128         # 12 key chunks per core
MI = M // 128            # 16 query chunks

_CACHE = {}
LAST_EXEC_NS = None
LAST_TRACE_DIR = None


def _build():
    import concourse.bass as bass
    import concourse.tile as tile
    from concourse import bacc, mybir
    from collections import deque

    F32 = mybir.dt.float32
    FP16 = mybir.dt.float16
    BF16 = mybir.dt.bfloat16
    Exp = mybir.ActivationFunctionType.Exp

    nc = bacc.Bacc("TRN2", target_bir_lowering=False, debug=False)

    # ---- I/O (layouts chosen so every DMA is contiguous) ----
    xt_m = nc.dram_tensor("xt_m", [128, 4, EC, 512], FP16, kind="ExternalInput").ap()
    mbt = nc.dram_tensor("mbt", [HH * DK, M], FP16, kind="ExternalInput").ap()
    xt_e = nc.dram_tensor("xt_e", [128, 3, EC, 512], FP16, kind="ExternalInput").ap()
    ebt = nc.dram_tensor("ebt", [HH * DK, NLOC], FP16, kind="ExternalInput").ap()
    wq = nc.dram_tensor("wq", [128, PP, EC, 128], FP16, kind="ExternalInput").ap()
    wk = nc.dram_tensor("wk", [128, PP, EC, 128], FP16, kind="ExternalInput").ap()
    wv = nc.dram_tensor("wv", [128, EC, TT], FP16, kind="ExternalInput").ap()
    bq = nc.dram_tensor("bq", [128, PP], F32, kind="ExternalInput").ap()
    out_p = nc.dram_tensor("out_p", [HH, M, DK + 1], F32, kind="ExternalOutput").ap()

    with tile.TileContext(nc) as tc, ExitStack() as ctx:
        persist = ctx.enter_context(tc.tile_pool(name="persist", bufs=1))
        qpt_pool = ctx.enter_context(tc.tile_pool(name="qpt", bufs=5))
        attn_pool = ctx.enter_context(tc.tile_pool(name="attn", bufs=24))
        osb_pool = ctx.enter_context(tc.tile_pool(name="osb", bufs=16))
        proj_ps = ctx.enter_context(tc.tile_pool(name="proj_ps", bufs=2, space="PSUM"))
        sc_ps = ctx.enter_context(tc.tile_pool(name="sc_ps", bufs=2, space="PSUM"))
        av_ps = ctx.enter_context(tc.tile_pool(name="av_ps", bufs=2, space="PSUM"))

        # K'T per head [128, NLOC]: rows = k-dims | eb-dims (parity layout:
        # even head k at partitions 0:64, odd head k at 64:128 - avoids any
        # cross-partition copies; scores only need a consistent d' order)
        kpt = [persist.tile([128, NLOC], FP16, tag=f"kpt{h}", name=f"kpt{h}")
               for h in range(HH)]
        # V per key chunk [128, HH, DK+1] bf16, ones col at [., ., DK]
        vsb = [persist.tile([128, HH, DK + 1], BF16, tag=f"v{ni}", name=f"v{ni}")
               for ni in range(NI)]
        bq_sb = persist.tile([128, PP], F32, tag="bq")
        wk_big = persist.tile([128, PP, EC, 128], FP16, tag="wk")
        wq_big = persist.tile([128, PP, EC, 128], FP16, tag="wq")
        wv_big = persist.tile([128, EC, TT], FP16, tag="wv")
        xte_big = persist.tile([128, 3, EC, 512], FP16, tag="xte")
        xtm_big = persist.tile([128, 4, EC, 512], FP16, tag="xtm")
        q0_0 = qpt_pool.tile([128, M], FP16, tag="qpt", name="qpt0")
        q1_0 = qpt_pool.tile([128, M], FP16, tag="qpt", name="qpt1")

        # input DMAs: criticals (pair-0 / first key-third) on dedicated
        # queues, bulk remainder behind them
        nc.scalar.dma_start(bq_sb[:], bq)
        nc.sync.dma_start(wk_big[:, 0], wk[:, 0])
        nc.gpsimd.dma_start(q0_0[64:128, :], mbt[0:DK, :])
        nc.gpsimd.dma_start(q1_0[0:64, :], mbt[DK:2 * DK, :])
        nc.gpsimd.dma_start(wq_big[:, 0], wq[:, 0])
        nc.gpsimd.dma_start(kpt[0][64:128, :], ebt[0:DK, :])
        nc.gpsimd.dma_start(kpt[1][0:64, :], ebt[DK:2 * DK, :])
        # kt0's key-third split across sync+gpsimd so no single ~100GB/s
        # queue carries the whole 1.5MB on the critical path
        nc.sync.dma_start(xte_big[:, 0, 0:3], xt_e[:, 0, 0:3])
        nc.gpsimd.dma_start(xte_big[:, 0, 3:EC], xt_e[:, 0, 3:EC])
        nc.scalar.dma_start(xtm_big[:, 0], xt_m[:, 0])
        nc.scalar.dma_start(xtm_big[:, 1], xt_m[:, 1])
        nc.sync.dma_start(xte_big[:, 1], xt_e[:, 1])
        nc.gpsimd.dma_start(wv_big[:], wv[:])
        nc.scalar.dma_start(xtm_big[:, 2:4], xt_m[:, 2:4])
        nc.sync.dma_start(xte_big[:, 2], xt_e[:, 2])
        nc.gpsimd.dma_start(wk_big[:, 1:PP], wk[:, 1:PP])
        nc.gpsimd.dma_start(wq_big[:, 1:PP], wq[:, 1:PP])

        def emit_scores_exp_half(h, qt, ni, half, at):
            """scoresT half [128 keys, 1024 queries] + exp into attnT.
            [128,1024] scores psum tiles double-buffer (2 banks each) so
            ACT runs exp back-to-back with no psum-free wait."""
            ps = sc_ps.tile([128, 1024], F32, tag="sc", name="sc_ps_t")
            mo = half * 1024
            for mj in range(2):
                nc.tensor.matmul(
                    ps[:, mj * 512:(mj + 1) * 512],
                    lhsT=kpt[h][:, ni * 128:(ni + 1) * 128],
                    rhs=qt[:, mo + mj * 512:mo + (mj + 1) * 512],
                    start=True, stop=True)
            nc.scalar.activation(at[:, mo:mo + 1024], ps[:], Exp)

        def emit_av(h, attns, g):
            """out chunks [128 queries, DK+1], mi in [2g, 2g+2); the bf16
            attnT chunk is the stationary operand (fast weight load)."""
            for mi in range(2 * g, 2 * g + 2):
                ps = av_ps.tile([128, DK + 1], F32, tag="av", name="av_ps_t")
                for ni in range(NI):
                    nc.tensor.matmul(
                        ps[:], lhsT=attns[ni][:, mi * 128:(mi + 1) * 128],
                        rhs=vsb[ni][:, h, :],
                        start=(ni == 0), stop=(ni == NI - 1))
                ot = osb_pool.tile([128, DK + 1], F32, tag="osb", name="osb_t")
                nc.vector.tensor_copy(ot[:], ps[:])
                nc.sync.dma_start(out_p[h, mi * 128:(mi + 1) * 128, :], ot[:])

        # ---- unit-queue scheduler ----
        units = deque()
        qts = {0: q0_0, 1: q1_0}
        pieces = {0: set()}  # pair -> done piece ids (k0..k2, q0..q3)

        def emit_qt_quarter(p, mh, q0, q1):
            ps = proj_ps.tile([128, 512], F32, tag="proj", name="proj_qt")
            for ec in range(EC):
                nc.tensor.matmul(ps[:], lhsT=wq_big[:, p, ec, :],
                                 rhs=xtm_big[:, mh, ec, :],
                                 start=(ec == 0), stop=(ec == EC - 1))
            mo = mh * 512
            nc.vector.tensor_scalar_add(
                q0[0:64, mo:mo + 512], ps[0:64, :], bq_sb[0:64, p:p + 1])
            nc.vector.tensor_scalar_add(
                q1[64:128, mo:mo + 512], ps[64:128, :], bq_sb[64:128, p:p + 1])

        def qt_unit(p, mh):
            def f():
                pieces.setdefault(p, set()).add(f"q{mh}")
                q0, q1 = qts.get(2 * p), qts.get(2 * p + 1)
                if q0 is None:
                    q0 = qpt_pool.tile([128, M], FP16, tag="qpt", name=f"qpt{2*p}")
                    q1 = qpt_pool.tile([128, M], FP16, tag="qpt", name=f"qpt{2*p+1}")
                    h0, h1 = 2 * p, 2 * p + 1
                    nc.sync.dma_start(q0[64:128, :], mbt[h0 * DK:(h0 + 1) * DK, :])
                    nc.sync.dma_start(q1[0:64, :], mbt[h1 * DK:(h1 + 1) * DK, :])
                    qts[2 * p], qts[2 * p + 1] = q0, q1
                emit_qt_quarter(p, mh, q0, q1)
            return (1.35, f)

        def emit_kt_third(p, t):
            h0, h1 = 2 * p, 2 * p + 1
            lo = t * 512
            ps = proj_ps.tile([128, 512], F32, tag="proj", name="proj_kt")
            for ec in range(EC):
                nc.tensor.matmul(ps[:], lhsT=wk_big[:, p, ec, :],
                                 rhs=xte_big[:, t, ec, :],
                                 start=(ec == 0), stop=(ec == EC - 1))
            nc.vector.tensor_copy(kpt[h0][0:64, lo:lo + 512], ps[0:64, :])
            nc.vector.tensor_copy(kpt[h1][64:128, lo:lo + 512], ps[64:128, :])
            if t == 0 and p > 0:
                nc.sync.dma_start(kpt[h0][64:128, :], ebt[h0 * DK:(h0 + 1) * DK, :])
                nc.sync.dma_start(kpt[h1][0:64, :], ebt[h1 * DK:(h1 + 1) * DK, :])

        def kt_unit(p, t):
            def f():
                pieces.setdefault(p, set()).add(f"k{t}")
                emit_kt_third(p, t)
            return (1.35, f)

        def v_unit(ni):
            def f():
                ps = proj_ps.tile([128, TT], F32, tag="proj", name="proj_v")
                t, off = divmod(ni, 4)
                for ec in range(EC):
                    nc.tensor.matmul(
                        ps[:], lhsT=xte_big[:, t, ec, off * 128:(off + 1) * 128],
                        rhs=wv_big[:, ec, :], start=(ec == 0), stop=(ec == EC - 1))
                nc.vector.tensor_copy(
                    vsb[ni][:, :, 0:DK], ps[:].rearrange("p (h d) -> p h d", d=DK))
                nc.vector.memset(vsb[ni][:, :, DK], 1.0)
            return (1.0, f)

        def av_unit(h, attns, g):
            def f():
                emit_av(h, attns, g)
            return (0.8, f)

        def pump(budget):
            while units and budget > 0:
                c, f = units.popleft()
                f()
                budget -= c

        # minimal head-0 critical path up front: kt third0 + qt q0/q1
        emit_kt_third(0, 0)
        pieces[0].add("k0")
        qt_unit(0, 0)[1]()
        qt_unit(0, 1)[1]()
        units.append(kt_unit(0, 1))
        units.append(kt_unit(0, 2))
        units.append(qt_unit(0, 2))
        units.append(qt_unit(0, 3))
        for ni in range(NI):
            units.append(v_unit(ni))

        def need(p, ni, half):
            req = {f"k{ni // 4}", f"q{2 * half}", f"q{2 * half + 1}"}
            while not req <= pieces.get(p, set()):
                c, f = units.popleft()
                f()

        slot = 0
        for h in range(HH):
            p = h // 2
            if h % 2 == 1 and p + 1 <= PP - 1:
                # next pair's projections jump the queue (front) so the
                # even-head boundary never force-drains a big batch
                for mh in range(3, -1, -1):
                    units.appendleft(qt_unit(p + 1, mh))
                for t in range(2, -1, -1):
                    units.appendleft(kt_unit(p + 1, t))
            attns = [attn_pool.tile([128, M], BF16, tag="attn",
                                    name=f"attn_{h}_{ni}") for ni in range(NI)]
            for half in range(2):
                for ni in range(NI):
                    need(p, ni, half)
                    emit_scores_exp_half(h, qts[h], ni, half, attns[ni])
                    pump(2.0 if slot < 24 else 0.75)
                    slot += 1
                # av groups for mi chunks covered by this half can go
                # into the queue now (g<4 reads attnT cols 0:1024 only)
                gs = range(4) if half == 0 else range(4, 8)
                for g in gs:
                    units.append(av_unit(h, attns, g))
            qts[h] = None  # release the qpt slot
        while units:
            c, f = units.popleft()
            f()

    nc.compile()
    return nc


def _get_nc():
    if "nc" not in _CACHE:
        _CACHE["nc"] = _build()
    return _CACHE["nc"]


def kernel(**inputs):
    global LAST_EXEC_NS, LAST_TRACE_DIR
    from concourse.bass_utils import run_bass_kernel_spmd

    ehr = np.asarray(inputs["ehr_embeddings"], dtype=np.float32)
    mi = np.asarray(inputs["missing_indices"]).astype(np.int64)
    ei = np.asarray(inputs["exist_indices"]).astype(np.int64)
    Wq = np.asarray(inputs["Wq"], dtype=np.float32)
    Wk = np.asarray(inputs["Wk"], dtype=np.float32)
    Wv = np.asarray(inputs["Wv"], dtype=np.float32)
    bq = np.asarray(inputs["bq"], dtype=np.float32)
    bv = np.asarray(inputs["bv"], dtype=np.float32)
    cooc = np.asarray(inputs["cooc_bias"], dtype=np.float32)

    scale = 1.0 / np.sqrt(np.float32(DK))

    def fold(a):  # [E, F] -> [128, EC, F]
        return a.reshape(EC, 128, a.shape[1]).transpose(1, 0, 2)

    def wfold(a):  # [E, TT] -> [128, PP, EC, 128] (pair-col major)
        return np.ascontiguousarray(
            fold(a).reshape(128, EC, PP, 128).transpose(0, 2, 1, 3))

    missing_emb = ehr[mi]                       # [M, E]
    xt_m = np.ascontiguousarray(
        fold(missing_emb.T.astype(np.float16))
        .reshape(128, EC, 4, 512).transpose(0, 2, 1, 3))  # [128, 4, EC, 512]
    wq_all = (Wq * scale).astype(np.float16)
    wk_all = Wk.astype(np.float16)
    wv_all = Wv.astype(np.float16)
    mbt_all = cooc[:, mi, :].transpose(0, 2, 1).reshape(H * DK, M).astype(np.float16)
    bq_all = (bq * scale).astype(np.float32)

    in_maps = []
    for c in range(CORES):
        hg, ns = c // NSHARDS, c % NSHARDS
        hsl = slice(hg * TT, (hg + 1) * TT)
        eic = ei[ns * NLOC:(ns + 1) * NLOC]
        xte_f = fold(ehr[eic].T.astype(np.float16))  # [128, EC, NLOC]
        xt_e = np.ascontiguousarray(
            xte_f.reshape(128, EC, 3, 512).transpose(0, 2, 1, 3))
        ebt = np.ascontiguousarray(
            cooc[hg * HH:(hg + 1) * HH, eic, :].transpose(0, 2, 1)
            .reshape(HH * DK, NLOC).astype(np.float16))
        in_maps.append({
            "xt_m": xt_m,
            "mbt": np.ascontiguousarray(mbt_all[hsl]),
            "xt_e": xt_e, "ebt": ebt,
            "wq": wfold(wq_all[:, hsl]),
            "wk": wfold(wk_all[:, hsl]),
            "wv": np.ascontiguousarray(fold(wv_all[:, hsl])),
            "bq": np.ascontiguousarray(bq_all[hsl].reshape(PP, 128).T),
        })

    nc = _get_nc()
    kwargs = {}
    if os.environ.get("KERNEL_TRACE") == "1":
        import tempfile
        LAST_TRACE_DIR = tempfile.mkdtemp(prefix="kern_trace_")
        kwargs = {"trace": True, "tmpdir": LAST_TRACE_DIR}
        try:
            import ntff_shim
            ntff_shim.install()
        except ImportError:
            pass
    res = run_bass_kernel_spmd(nc, in_maps, list(range(CORES)), **kwargs)
    LAST_EXEC_NS = res.exec_time_ns

    # ---- host combine (exact softmax across the 4 key shards) ----
    num = np.zeros((H, M, DK), dtype=np.float64)
    den = np.zeros((H, M), dtype=np.float64)
    for c in range(CORES):
        hg = c // NSHARDS
        op = res.results[c]["out_p"].astype(np.float64)  # [HH, M, DK+1]
        num[hg * HH:(hg + 1) * HH] += op[:, :, :DK]
        den[hg * HH:(hg + 1) * HH] += op[:, :, DK]
    out = num / den[:, :, None]                          # [H, M, DK]
    out = out.transpose(1, 0, 2).reshape(M, TOTAL) + bv.astype(np.float64)
    result = ehr.copy()
    result[mi] = out.astype(np.float32)
    return result
